# revision 1
# baseline (speedup 1.0000x reference)
"""Causal self-attention (B=2, T=2048, C=768, H=12) on 8 NeuronCores.

Sharding: zero-collective B x query-chunk sharding. Core d handles batch
b = d//4 and the causally-balanced query-chunk pair j = d%4:
rows [256j, 256j+256) and [256(7-j), 256(7-j)+256). Each core computes
K/V for the full sequence locally (redundant but collective-free), all 12
heads for its 512 query rows, and the output projection for its rows.
Output gather on host is a pure concat.

On-device pipeline (bf16 matmul operands, fp32 PSUM accumulation):
  xT = PE-transpose(x)            ->  Q^T/K^T via W-stationary matmuls
  S^T[k,q] = K^T(h)^T Q^T(h)      ->  exp on ACT (scale=1/8, no max-sub;
  |logits| <= ~20 so f32 exp is safe) -> causal mask multiply (host-built
  mask tiles) -> P^T @ [V|1] folds the softmax denominator into row 64 of
  the PSUM accumulator -> reciprocal + K=1 ones-matmul broadcast ->
  normalize into Y^T -> output projection + bias.

Engine discipline: ACT runs only Exp (activation-table swaps are
expensive); every PSUM->SBUF copy-out runs on DVE. Steps 0-7 process
both query chunks in one fused [128,512] tile; steps 8-15 are the
chunk-B tail.
"""

import numpy as np
import ml_dtypes

B, T, C, H, D = 2, 2048, 768, 12, 64
NCORES = 8
QCH = 256          # query chunk width
NSTEPS_A, NSTEPS_B = 8, 16   # uniform k-tile extents for chunk A / B

_CACHE = {}


def _build_program(with_bias=True):
    import os
    KOPT = set(os.environ.get("KOPT", "").split(","))
    import concourse.bass as bass
    import concourse.bacc as bacc
    import concourse.mybir as mybir
    import concourse.tile as tile

    F32 = mybir.dt.float32
    BF16 = mybir.dt.bfloat16
    AF = mybir.ActivationFunctionType

    nc = bacc.Bacc()
    xbf = nc.declare_dram_parameter("xbf", [T, C], BF16, isOutput=False)
    xqbf = nc.declare_dram_parameter("xqbf", [2 * QCH, C], BF16, isOutput=False)
    wqkv = nc.declare_dram_parameter("wqkv", [C, 3 * C], BF16, isOutput=False)
    wproj = nc.declare_dram_parameter("wproj", [C, C], BF16, isOutput=False)
    bqkv = nc.declare_dram_parameter("bqkv", [128, 18], F32, isOutput=False)
    bproj = nc.declare_dram_parameter("bproj", [1, C], F32, isOutput=False)
    masks = nc.declare_dram_parameter("masks", [128, NSTEPS_B, 2 * QCH], BF16,
                                      isOutput=False)
    ident_in = nc.declare_dram_parameter("ident_in", [128, 128], BF16,
                                         isOutput=False)
    z_out = nc.declare_dram_parameter("z", [2 * QCH, C], F32, isOutput=True)

    CT = C // 128            # 6 c-tiles
    TT = T // 128            # 16 t-tiles
    QT_N = 2 * QCH           # 512 own query rows

    with tile.TileContext(nc) as tc:
        with tc.tile_pool(name="const", bufs=1) as constp, \
             tc.tile_pool(name="data", bufs=1) as datap, \
             tc.tile_pool(name="work", bufs=2) as workp, \
             tc.tile_pool(name="pt", bufs=6) as ptp, \
             tc.tile_pool(name="small", bufs=2) as smallp, \
             tc.tile_pool(name="zs", bufs=2) as zsp, \
             tc.tile_pool(name="ps", bufs=int(os.environ.get("KPS", "2")),
                          space="PSUM") as psp, \
             tc.tile_pool(name="pso", bufs=2, space="PSUM") as psop:

            # ---- constants (declared; DMAs emitted in priority order) -----
            masks_s = constp.tile([128, NSTEPS_B, 2 * QCH], BF16, tag="masks")
            bqkv_s = constp.tile([128, 18], F32, tag="bqkv")
            bpb = constp.tile([128, C], F32, tag="bpb")
            ones1 = constp.tile([1, 64], BF16, tag="ones1")
            nc.vector.memset(ones1, 1.0)
            wq_s = constp.tile([128, CT, 3 * C], BF16, tag="wqkv")
            wp_s = constp.tile([128, CT, C], BF16, tag="wproj")
            idn = constp.tile([128, 128], BF16, tag="idn")
            w_ap = wqkv[:, :]
            wp_ap = wproj[:, :]
            bp_ap = bproj[:, :]

            # ---- persistent tiles -----------------------------------------
            xT = [datap.tile([128, T], BF16, tag=f"xT{c}", name=f"xT{c}")
                  for c in range(CT)]
            xqT = [datap.tile([128, QT_N], BF16, tag=f"xqT{c}", name=f"xqT{c}")
                   for c in range(CT)]
            KT = [datap.tile([128, T], BF16, tag=f"KT{m}", name=f"KT{m}")
                  for m in range(CT)]
            QTt = [datap.tile([128, QT_N], BF16, tag=f"QT{m}", name=f"QT{m}")
                   for m in range(CT)]
            V = [datap.tile([128, H, D + 1], BF16, tag=f"V{t}", name=f"V{t}")
                 for t in range(TT)]
            YT = [datap.tile([128, QT_N], BF16, tag=f"YT{m}", name=f"YT{m}")
                  for m in range(CT)]

            # ---- phase A: hybrid transposes, priority-ordered DMAs --------
            # sync queue: idn -> x c-block quarters 0-2 -> W(K) by m-col
            # scalar queue: xbar transposes xT 3-5, xq 0-5, then W(V), W(Q)
            # gpsimd SWDGE queue: bqkv, bproj, Wproj, masks (non-critical)
            nc.sync.dma_start(out=idn, in_=ident_in[:, :])
            for c in range(3, CT):
                nc.scalar.dma_start_transpose(
                    out=xT[c], in_=xbf[:, 128 * c:128 * (c + 1)])
            for c in range(CT):
                nc.scalar.dma_start_transpose(
                    out=xqT[c], in_=xqbf[:, 128 * c:128 * (c + 1)])
            xls = []
            for c in range(3):
                xl = workp.tile([128, TT, 128], BF16, tag=f"xld{c}",
                                name=f"xld{c}")
                nc.sync.dma_start(
                    out=xl,
                    in_=bass.AP(tensor=xbf[:, :].tensor,
                                offset=xbf[:, :].offset + 128 * c,
                                ap=[[C, 128], [128 * C, TT], [1, 128]]),
                )
                xls.append(xl)
            for (lo, hi) in ((2 * C, 2 * C + 512), (2 * C + 512, 3 * C)):
                nc.sync.dma_start(
                    out=wq_s[:, :, lo:hi],
                    in_=bass.AP(tensor=w_ap.tensor, offset=w_ap.offset + lo,
                                ap=[[3 * C, 128], [128 * 3 * C, CT],
                                    [1, hi - lo]]),
                )
            for m in range(CT):
                nc.gpsimd.dma_start(
                    out=wq_s[:, :, C + 128 * m:C + 128 * (m + 1)],
                    in_=bass.AP(tensor=w_ap.tensor,
                                offset=w_ap.offset + C + 128 * m,
                                ap=[[3 * C, 128], [128 * 3 * C, CT],
                                    [1, 128]]),
                )
            for m in range(CT):
                nc.gpsimd.dma_start(
                    out=wq_s[:, :, 128 * m:128 * (m + 1)],
                    in_=bass.AP(tensor=w_ap.tensor,
                                offset=w_ap.offset + 128 * m,
                                ap=[[3 * C, 128], [128 * 3 * C, CT],
                                    [1, 128]]),
                )
            if with_bias:
                nc.gpsimd.dma_start(out=bqkv_s, in_=bqkv[:, :])
                nc.gpsimd.dma_start(
                    out=bpb,
                    in_=bass.AP(tensor=bp_ap.tensor, offset=bp_ap.offset,
                                ap=[[0, 128], bp_ap.ap[1]]),
                )
            nc.gpsimd.dma_start(
                out=wp_s,
                in_=bass.AP(tensor=wp_ap.tensor, offset=wp_ap.offset,
                            ap=[[C, 128], [128 * C, CT], wp_ap.ap[1]]),
            )
            nc.gpsimd.dma_start(out=masks_s, in_=masks[:, :, :])
            for c in range(3):
                for t in range(TT):
                    tp = psp.tile([128, 128], BF16, tag="s", name="tp")
                    nc.tensor.transpose(out=tp, in_=xls[c][:, t, :],
                                        identity=idn)
                    nc.vector.tensor_copy(
                        out=xT[c][:, 128 * t:128 * (t + 1)], in_=tp)

            # ---- phases B+C interleaved -----------------------------------
            # V first (needed by every pair); then per pair m: KT(m), QT(m),
            # the pair's attention steps, and the deferred normalize of the
            # previous pair. Keeps PE fed during the ACT-heavy attention.
            for t in range(TT):
                for (off, w) in ((0, 512), (512, 256)):
                    acc = psp.tile([128, 512], F32, tag="acc", name="acc")
                    corder = (3, 4, 5, 0, 1, 2)
                    for ci, c in enumerate(corder):
                        nc.tensor.matmul(
                            out=acc[:, 0:w],
                            lhsT=xT[c][:, 128 * t:128 * (t + 1)],
                            rhs=wq_s[:, c, 2 * C + off:2 * C + off + w],
                            start=(ci == 0), stop=(ci == CT - 1))
                    h0 = off // D
                    nc.vector.tensor_copy(
                        out=V[t][:, h0:h0 + w // D, 0:D], in_=acc[:, 0:w])
                nc.vector.memset(V[t][:, :, D:D + 1], 1.0)

            scale = 1.0 / float(np.sqrt(D))

            def normalize_pair(m, ots):
                for i in range(2):
                    po = 64 * i
                    ot = ots[i]
                    for qc in range(2):
                        csl = slice(QCH * qc, QCH * (qc + 1))
                        rec = smallp.tile([1, QCH], F32, tag="rec",
                                          name="rec")
                        nc.vector.reciprocal(out=rec, in_=ot[64:65, csl])
                        recbf = smallp.tile([1, QCH], BF16, tag="recbf",
                                            name="recbf")
                        nc.vector.tensor_copy(out=recbf, in_=rec)
                        recb = psp.tile([64, QCH], F32, tag="acc",
                                        name="recb")
                        nc.tensor.matmul(out=recb, lhsT=ones1, rhs=recbf,
                                         start=True, stop=True)
                        recb_sb = smallp.tile([64, QCH], F32,
                                              tag="recb_sb", name="recb_sb")
                        nc.vector.tensor_copy(out=recb_sb, in_=recb)
                        ysl = YT[m][po:po + 64, csl]
                        nc.vector.tensor_mul(ysl, ot[0:64, csl], recb_sb)
                        if with_bias:
                            nc.vector.tensor_scalar_add(
                                ysl, in0=ysl,
                                scalar1=bqkv_s[po:po + 64, 12 + m:13 + m])

            pending = None
            for m in range(CT):
                for n in range(T // 512):
                    acc = psp.tile([128, 512], F32, tag="acc", name="acc")
                    for c in range(CT):
                        nc.tensor.matmul(
                            out=acc,
                            lhsT=wq_s[:, c, C + 128 * m:C + 128 * (m + 1)],
                            rhs=xT[c][:, 512 * n:512 * (n + 1)],
                            start=(c == 0), stop=(c == CT - 1))
                    if with_bias:
                        nc.vector.tensor_scalar_add(
                            KT[m][:, 512 * n:512 * (n + 1)], in0=acc,
                            scalar1=bqkv_s[:, 6 + m:7 + m])
                    else:
                        nc.vector.tensor_copy(
                            out=KT[m][:, 512 * n:512 * (n + 1)], in_=acc)
                acc = psp.tile([128, 512], F32, tag="acc", name="acc")
                for c in range(CT):
                    nc.tensor.matmul(
                        out=acc,
                        lhsT=wq_s[:, c, 128 * m:128 * (m + 1)],
                        rhs=xqT[c],
                        start=(c == 0), stop=(c == CT - 1))
                if with_bias:
                    nc.vector.tensor_scalar_add(
                        QTt[m], in0=acc, scalar1=bqkv_s[:, m:m + 1])
                else:
                    nc.vector.tensor_copy(out=QTt[m], in_=acc)

                ots = [psop.tile([65, 2 * QCH], F32, tag=f"ot{i}",
                                 name=f"ot{i}") for i in range(2)]
                for s in range(NSTEPS_B):
                    w = 512 if s < NSTEPS_A else 256
                    qsl = (slice(0, 512) if s < NSTEPS_A
                           else slice(QCH, 2 * QCH))
                    pts = []
                    for i, po in ((0, 0), (1, 64)):
                        sps = psp.tile([128, 512], F32, tag="s", name="sps")
                        nc.tensor.matmul(
                            out=sps[:, 0:w],
                            lhsT=KT[m][po:po + 64, 128 * s:128 * (s + 1)],
                            rhs=QTt[m][po:po + 64, qsl],
                            start=True, stop=True)
                        pt = ptp.tile([128, 512], BF16, tag="pt", name="pt")
                        nc.scalar.activation(out=pt[:, 0:w], in_=sps[:, 0:w],
                                             func=AF.Exp, scale=scale)
                        nc.vector.tensor_mul(pt[:, 0:QCH], pt[:, 0:QCH],
                                             masks_s[:, s, 0:QCH])
                        pts.append(pt)
                    for i in range(2):
                        h = 2 * m + i
                        if s < NSTEPS_A:
                            nc.tensor.matmul(
                                out=ots[i], lhsT=V[s][:, h, :],
                                rhs=pts[i][:, 0:512],
                                start=(s == 0), stop=False,
                                skip_group_check=True)
                        else:
                            nc.tensor.matmul(
                                out=ots[i][:, QCH:2 * QCH],
                                lhsT=V[s][:, h, :], rhs=pts[i][:, 0:QCH],
                                start=False, stop=(s == NSTEPS_B - 1),
                                skip_group_check=True)
                if pending is not None:
                    normalize_pair(*pending)
                pending = (m, ots)
            normalize_pair(*pending)

            # ---- phase D: projection (two f-half groups per output) -------
            for qm in range(QT_N // 128):
                for (off, w) in ((0, 512), (512, 256)):
                    acc1 = psp.tile([128, 512], F32, tag="acc", name="acc1")
                    for f in range(3):
                        nc.tensor.matmul(
                            out=acc1[:, 0:w],
                            lhsT=YT[f][:, 128 * qm:128 * (qm + 1)],
                            rhs=wp_s[:, f, off:off + w],
                            start=(f == 0), stop=(f == 2))
                    acc2 = psp.tile([128, 512], F32, tag="acc", name="acc2")
                    for f in range(3, CT):
                        nc.tensor.matmul(
                            out=acc2[:, 0:w],
                            lhsT=YT[f][:, 128 * qm:128 * (qm + 1)],
                            rhs=wp_s[:, f, off:off + w],
                            start=(f == 3), stop=(f == CT - 1))
                    zt1 = zsp.tile([128, 512], F32, tag="zt1", name="zt1")
                    nc.vector.tensor_copy(out=zt1[:, 0:w], in_=acc1[:, 0:w])
                    zt = zsp.tile([128, 512], F32, tag="zt", name="zt")
                    nc.vector.tensor_add(zt[:, 0:w], acc2[:, 0:w],
                                         zt1[:, 0:w])
                    if with_bias:
                        nc.vector.tensor_add(zt[:, 0:w], zt[:, 0:w],
                                             bpb[:, off:off + w])
                    nc.sync.dma_start(
                        out=z_out[128 * qm:128 * (qm + 1), off:off + w],
                        in_=zt[:, 0:w])

    nc.finalize()
    return nc


def _prep_inputs(x, W_qkv, b_qkv, W_proj, b_proj):
    bf16 = ml_dtypes.bfloat16
    x = np.ascontiguousarray(np.asarray(x, dtype=np.float32))
    W_qkv = np.asarray(W_qkv, dtype=np.float32)
    b_qkv = np.asarray(b_qkv, dtype=np.float32)
    W_proj = np.asarray(W_proj, dtype=np.float32)
    b_proj = np.asarray(b_proj, dtype=np.float32)

    wqkv_b = np.ascontiguousarray(W_qkv.astype(bf16))
    wproj_b = np.ascontiguousarray(W_proj.astype(bf16))
    # b_qkv [2304] -> [128, 18] with [p, m] = b[128m + p]
    bqkv_t = np.ascontiguousarray(b_qkv.reshape(18, 128).T)
    bproj_t = np.ascontiguousarray(b_proj.reshape(1, C))
    xb = [np.ascontiguousarray(x[b].astype(bf16)) for b in range(B)]

    in_maps = []
    p = np.arange(128)
    for d in range(NCORES):
        b, j = d // 4, d % 4
        rows_a = np.arange(QCH * j, QCH * (j + 1))
        rows_b = np.arange(QCH * (7 - j), QCH * (8 - j))
        qrows = np.concatenate([rows_a, rows_b])
        xq = np.ascontiguousarray(xb[b][qrows])
        # masks [128, 16, 512]: steps 0-7 cols 0:256 = chunk A ktile s,
        # cols 256:512 = chunk B ktile s (all ones); steps 8-15 cols
        # 0:256 = chunk B ktile s (tail), cols 256:512 unused (ones).
        m = np.ones((128, NSTEPS_B, 2 * QCH), dtype=np.float32)
        for s in range(NSTEPS_B):
            kabs = (128 * s + p)[:, None]
            if s < NSTEPS_A:
                m[:, s, 0:QCH] = rows_a[None, :] >= kabs
                m[:, s, QCH:] = rows_b[None, :] >= kabs
            else:
                m[:, s, 0:QCH] = rows_b[None, :] >= kabs
        in_maps.append({
            "xbf": xb[b],
            "xqbf": xq,
            "wqkv": wqkv_b,
            "wproj": wproj_b,
            "bqkv": bqkv_t,
            "bproj": bproj_t,
            "masks": np.ascontiguousarray(m.astype(bf16)),
            "ident_in": np.eye(128, dtype=np.float32).astype(bf16),
        })
    return in_maps


def kernel(x, W_qkv, b_qkv, W_proj, b_proj):
    import os
    from concourse.bass_utils import run_bass_kernel_spmd

    in_maps = _prep_inputs(x, W_qkv, b_qkv, W_proj, b_proj)
    with_bias = bool(np.any(np.asarray(b_qkv)) or np.any(np.asarray(b_proj)))
    key = f"nc{with_bias}"
    if key not in _CACHE:
        _CACHE[key] = _build_program(with_bias)
    nc = _CACHE[key]
    res = run_bass_kernel_spmd(nc, in_maps, list(range(NCORES)),
                               trace=os.environ.get("KTRACE", "") == "1")
    _CACHE["last_result"] = res

    out = np.empty((B, T, C), dtype=np.float32)
    for d in range(NCORES):
        b, j = d // 4, d % 4
        z = np.asarray(res.results[d]["z"])
        out[b, QCH * j:QCH * (j + 1)] = z[0:QCH]
        out[b, QCH * (7 - j):QCH * (8 - j)] = z[QCH:2 * QCH]
    return out



# revision 8
# speedup vs baseline: 1.6425x; 1.6425x over previous
"""Causal self-attention (B=2, T=2048, C=768, H=12) on 8 NeuronCores.

Sharding: batch x head-group tensor parallelism (per the hint). Core
d = 4b + g handles batch b and heads [3g, 3g+2]. Each core computes its
3 heads' Q/K/V for the full sequence, the full causal TxT attention for
those heads, and a PARTIAL output projection z_partial = Y_g @ Wproj[g
rows]. The host gather sums the 4 partials per batch (the TP unshard) --
zero device collectives.

On-device pipeline (bf16 operands, fp32 PSUM):
  x^T arrives pre-transposed from host -> QKV^T via W-stationary matmuls
  (K^T/Q^T head-pairs packed 2x64 per 128-partition tile; the odd head's
  K/Q pair is split to base-partition-0 tiles via an SBUF->SBUF DMA).
  S^T[k,q] = (K^T)^T Q^T per 128-k-tile with exact causal q-extent ->
  exp on ACT (scale=1/8, |logits|<=~20 so no max-subtraction) -> upper-
  triangular mask multiply on the diagonal 128-block only -> P.V flipped:
  out O[q,65] per (k-tile, q-tile) with lhsT = P^T block (stationary) and
  rhs = [V|1] -- 65-column outputs make PV ~2x cheaper than the [65,q]
  orientation, and the softmax denominator lands in column 64 so
  normalization is a per-partition broadcast multiply on DVE. Y is then
  PE-transposed per 128-tile for the output projection.

Engine budget per core (cost model): PE ~68us busy, ACT (exp only)
~57us, DVE ~50us, serial-DMA ~25us.
"""

import numpy as np
import ml_dtypes

B, T, C, H, D = 2, 2048, 768, 12, 64
NCORES = 8
HPG = 3              # heads per group (core)
CT = C // 128        # 6 channel tiles
TT = T // 128        # 16 seq tiles

_CACHE = {}


def _build_program(with_bias=True):
    import concourse.bass as bass
    import concourse.bacc as bacc
    import concourse.mybir as mybir
    import concourse.tile as tile

    F32 = mybir.dt.float32
    BF16 = mybir.dt.bfloat16
    AF = mybir.ActivationFunctionType

    nc = bacc.Bacc()
    xTd = nc.declare_dram_parameter("xT", [C, T], BF16, isOutput=False)
    wA = nc.declare_dram_parameter("wA", [C, 128], BF16, isOutput=False)
    wB = nc.declare_dram_parameter("wB", [C, 128], BF16, isOutput=False)
    wC = nc.declare_dram_parameter("wC", [C, 128], BF16, isOutput=False)
    wV = nc.declare_dram_parameter("wV", [C, HPG * D], BF16, isOutput=False)
    wp01 = nc.declare_dram_parameter("wp01", [128, C], BF16, isOutput=False)
    wp2 = nc.declare_dram_parameter("wp2", [64, C], BF16, isOutput=False)
    tri_in = nc.declare_dram_parameter("tri", [128, 128], BF16, isOutput=False)
    idn_in = nc.declare_dram_parameter("idn", [128, 128], BF16, isOutput=False)
    if with_bias:
        bqk = nc.declare_dram_parameter("bqk", [128, 3], F32, isOutput=False)
        bVd = nc.declare_dram_parameter("bV", [1, HPG * D], BF16, isOutput=False)
    z_out = nc.declare_dram_parameter("z", [T, C], BF16, isOutput=True)

    scale = 1.0 / float(np.sqrt(D))

    with tile.TileContext(nc) as tc:
        with tc.tile_pool(name="const", bufs=1) as constp, \
             tc.tile_pool(name="data", bufs=1) as datap, \
             tc.tile_pool(name="pt", bufs=3) as ptp, \
             tc.tile_pool(name="small", bufs=2) as smallp, \
             tc.tile_pool(name="zs", bufs=3) as zsp, \
             tc.tile_pool(name="po", bufs=4, space="PSUM") as pop, \
             tc.tile_pool(name="ps", bufs=2, space="PSUM") as psp:

            # ---- constant / weight tiles ---------------------------------
            idn = constp.tile([128, 128], BF16, tag="idn")
            tri = constp.tile([128, 128], BF16, tag="tri")
            wA_s = constp.tile([128, CT, 128], BF16, tag="wA")
            wB_s = constp.tile([128, CT, 128], BF16, tag="wB")
            wC_s = constp.tile([128, CT, 128], BF16, tag="wC")
            wV_s = constp.tile([128, CT, HPG * D], BF16, tag="wV")
            wp01_s = constp.tile([128, C], BF16, tag="wp01")
            wp2_s = constp.tile([64, C], BF16, tag="wp2")
            if with_bias:
                bqk_s = constp.tile([128, 3], F32, tag="bqk")
                bV_s = constp.tile([1, HPG * D], BF16, tag="bV")
                onesB = constp.tile([1, 128], BF16, tag="onesB")
                nc.vector.memset(onesB, 1.0)

            # ---- persistent data tiles -----------------------------------
            xT = [datap.tile([128, T], BF16, tag=f"xT{c}", name=f"xT{c}")
                  for c in range(CT)]
            KT01 = datap.tile([128, T], BF16, tag="KT01", name="KT01")
            QT01 = datap.tile([128, T], BF16, tag="QT01", name="QT01")
            KQ2 = datap.tile([128, T], BF16, tag="KQ2", name="KQ2")
            Q2 = datap.tile([64, T], BF16, tag="Q2", name="Q2")
            V = datap.tile([128, TT, HPG, D + 1], BF16, tag="V", name="V")
            Ybf01 = datap.tile([128, TT, 128], BF16, tag="Ybf01", name="Ybf01")
            Ybf2 = datap.tile([128, TT, D], BF16, tag="Ybf2", name="Ybf2")
            YT01 = datap.tile([128, T], BF16, tag="YT01", name="YT01")
            YT2 = datap.tile([64, T], BF16, tag="YT2", name="YT2")
            rec = datap.tile([128, TT, HPG], F32, tag="rec", name="rec")

            # ---- DMAs (ordered by criticality) ---------------------------
            nc.sync.dma_start(out=idn, in_=idn_in[:, :])
            nc.sync.dma_start(out=tri, in_=tri_in[:, :])
            for (w_s, w_d, wid) in ((wC_s, wC, 128), (wA_s, wA, 128)):
                ap = w_d[:, :]
                nc.sync.dma_start(
                    out=w_s,
                    in_=bass.AP(tensor=ap.tensor, offset=ap.offset,
                                ap=[[wid, 128], [128 * wid, CT], [1, wid]]))
            # x^T in halves so the first QKV chunks start early
            for c in range(CT):
                for hh in range(2):
                    nc.sync.dma_start(
                        out=xT[c][:, 1024 * hh:1024 * (hh + 1)],
                        in_=xTd[128 * c:128 * (c + 1),
                                1024 * hh:1024 * (hh + 1)])
            for (w_s, w_d, wid) in ((wB_s, wB, 128), (wV_s, wV, HPG * D)):
                ap = w_d[:, :]
                nc.gpsimd.dma_start(
                    out=w_s,
                    in_=bass.AP(tensor=ap.tensor, offset=ap.offset,
                                ap=[[wid, 128], [128 * wid, CT], [1, wid]]))
            nc.gpsimd.dma_start(out=wp01_s, in_=wp01[:, :])
            nc.gpsimd.dma_start(out=wp2_s, in_=wp2[:, :])
            if with_bias:
                nc.gpsimd.dma_start(out=bqk_s, in_=bqk[:, :])
                nc.gpsimd.dma_start(out=bV_s, in_=bVd[:, :])

            # ---- QKV^T projection ----------------------------------------
            def proj_128(dst, w_s, bcol):
                """dst[128, T] = (x @ W_block)^T, W_block stationary."""
                for n in range(T // 512):
                    acc = psp.tile([128, 512], F32, tag="S", name="acc")
                    for c in range(CT):
                        nc.tensor.matmul(
                            out=acc,
                            lhsT=w_s[:, c, :],
                            rhs=xT[c][:, 512 * n:512 * (n + 1)],
                            start=(c == 0), stop=(c == CT - 1))
                    if with_bias:
                        nc.vector.tensor_scalar_add(
                            dst[:, 512 * n:512 * (n + 1)], in0=acc,
                            scalar1=bqk_s[:, bcol:bcol + 1])
                    else:
                        nc.vector.tensor_copy(
                            out=dst[:, 512 * n:512 * (n + 1)],
                            in_=acc)

            proj_128(QT01, wC_s, 2)
            proj_128(KT01, wA_s, 0)

            # ---- V projection: V[t, 3, 64] + ones column -----------------
            nc.vector.memset(V[:, :, :, D:D + 1], 1.0)
            for t in range(TT):
                acc = psp.tile([128, 512], F32, tag="S", name="vacc")
                for c in range(CT):
                    nc.tensor.matmul(
                        out=acc[:, 0:HPG * D],
                        lhsT=xT[c][:, 128 * t:128 * (t + 1)],
                        rhs=wV_s[:, c, :],
                        start=(c == 0), stop=(with_bias is False and
                                              c == CT - 1))
                if with_bias:
                    nc.tensor.matmul(
                        out=acc[:, 0:HPG * D], lhsT=onesB, rhs=bV_s,
                        start=False, stop=True)
                nc.vector.tensor_copy(out=V[:, t, :, 0:D],
                                      in_=acc[:, 0:HPG * D])

            # KQ2 = [K^T h2 | Q^T h2] packed, then split Q half to Q2
            proj_128(KQ2, wB_s, 1)
            nc.sync.dma_start(out=Q2[0:64, :], in_=KQ2[64:128, :])

            # ---- attention per head --------------------------------------
            def attention(h):
                KTt, kpo = ((KT01, 0), (KT01, 64), (KQ2, 0))[h]
                QTt, qpo = ((QT01, 0), (QT01, 64), (Q2, 0))[h]
                obank = [None] * 4
                for s in range(TT):
                    E = T - 128 * s
                    if s == 0:
                        for b in range(4):
                            obank[b] = pop.tile([128, 4, 128], F32, tag="O",
                                                name=f"O{b}")
                    pt = ptp.tile([128, T], BF16, tag="pt", name="pt")
                    for w0 in range(0, E, 1024):
                        w = min(1024, E - w0)
                        sps = psp.tile([128, 1024], F32, tag="S", name="sps")
                        for half in range(0, w, 512):
                            hw = min(512, w - half)
                            nc.tensor.matmul(
                                out=sps[:, half:half + hw],
                                lhsT=KTt[kpo:kpo + 64, 128 * s:128 * (s + 1)],
                                rhs=QTt[qpo:qpo + 64,
                                        128 * s + w0 + half:
                                        128 * s + w0 + half + hw],
                                start=True, stop=True)
                        nc.scalar.activation(out=pt[:, w0:w0 + w],
                                             in_=sps[:, 0:w],
                                             func=AF.Exp, scale=scale)
                        if w0 == 0:
                            nc.vector.tensor_mul(pt[:, 0:128], pt[:, 0:128],
                                                 tri)
                        # PV for q-tiles covered by this chunk
                        t0 = s + w0 // 128
                        t1 = s + (w0 + w) // 128
                        for t in range(t0, t1):
                            ob = obank[t // 4]
                            # start=True zeroes the whole 2KB PSUM zero-region
                            # (bank); only the first window of each bank may
                            # issue it, siblings land on pending-zero bytes.
                            nc.tensor.matmul(
                                out=ob[:, t % 4, 0:D + 1],
                                lhsT=pt[:, 128 * (t - s):128 * (t - s) + 128],
                                rhs=V[:, s, h, :],
                                start=(s == 0 and t % 4 == 0), stop=(s == t),
                                skip_group_check=True)
                # normalize per bank: Y = O[:, :, 0:64] * (1/O[:, :, 64])
                for b in range(4):
                    ob = obank[b]
                    rsl = rec[:, 4 * b:4 * (b + 1), h:h + 1]
                    nc.vector.reciprocal(out=rsl, in_=ob[:, :, D:D + 1])
                    rb = bass.AP(tensor=rsl.tensor, offset=rsl.offset,
                                 ap=[rsl.ap[0], [HPG, 4], [0, D]])
                    if h < 2:
                        ydst = Ybf01[:, 4 * b:4 * (b + 1),
                                     64 * h:64 * h + D]
                    else:
                        ydst = Ybf2[:, 4 * b:4 * (b + 1), 0:D]
                    nc.vector.tensor_mul(ydst, ob[:, :, 0:D], rb)

            def transpose_head(h):
                for t in range(TT):
                    tp = psp.tile([128, 128], BF16, tag="S", name="tp")
                    if h < 2:
                        nc.tensor.transpose(out=tp, in_=Ybf01[:, t, :],
                                            identity=idn)
                        nc.vector.tensor_copy(
                            out=YT01[:, 128 * t:128 * (t + 1)], in_=tp)
                    else:
                        nc.tensor.transpose(out=tp[0:64, :],
                                            in_=Ybf2[:, t, :],
                                            identity=idn)
                        nc.vector.tensor_copy(
                            out=YT2[0:64, 128 * t:128 * (t + 1)],
                            in_=tp[0:64, :])

            attention(0)
            attention(1)
            transpose_head(0)   # h0/h1 share Ybf01 -> one transpose pass
            attention(2)
            transpose_head(2)

            # ---- output projection (partial z) ---------------------------
            for t in range(TT):
                zt = zsp.tile([128, C], BF16, tag="zt", name="zt")
                for (off, w) in ((0, 512), (512, 256)):
                    acc = pop.tile([128, 4, 128], F32, tag="O", name="zacc")
                    nc.tensor.matmul(
                        out=acc[:, 0:w // 128, :],
                        lhsT=YT01[:, 128 * t:128 * (t + 1)],
                        rhs=wp01_s[:, off:off + w],
                        start=True, stop=False)
                    nc.tensor.matmul(
                        out=acc[:, 0:w // 128, :],
                        lhsT=YT2[0:64, 128 * t:128 * (t + 1)],
                        rhs=wp2_s[:, off:off + w],
                        start=False, stop=True)
                    if t % 2 == 0:
                        nc.vector.tensor_copy(out=zt[:, off:off + w],
                                              in_=acc[:, 0:w // 128, :])
                    else:
                        nc.scalar.copy(out=zt[:, off:off + w],
                                       in_=acc[:, 0:w // 128, :])
                nc.sync.dma_start(
                    out=z_out[128 * t:128 * (t + 1), :], in_=zt)

    nc.finalize()
    return nc


def _prep_inputs(x, W_qkv, b_qkv, W_proj, b_proj):
    bf16 = ml_dtypes.bfloat16
    x = np.asarray(x, dtype=np.float32)
    W_qkv = np.asarray(W_qkv, dtype=np.float32)
    b_qkv = np.asarray(b_qkv, dtype=np.float32)
    W_proj = np.asarray(W_proj, dtype=np.float32)
    b_proj = np.asarray(b_proj, dtype=np.float32)

    Wq, Wk, Wv = W_qkv[:, 0:C], W_qkv[:, C:2 * C], W_qkv[:, 2 * C:3 * C]
    bq, bk, bv = b_qkv[0:C], b_qkv[C:2 * C], b_qkv[2 * C:3 * C]

    xTb = [np.ascontiguousarray(x[b].T.astype(bf16)) for b in range(B)]
    tri = np.triu(np.ones((128, 128), dtype=np.float32)).astype(bf16)
    idn = np.eye(128, dtype=np.float32).astype(bf16)

    in_maps = []
    for d in range(NCORES):
        b, g = d // 4, d % 4
        h0, h2 = 3 * g, 3 * g + 2
        m = {
            "xT": xTb[b],
            "wA": np.ascontiguousarray(Wk[:, 64 * h0:64 * h0 + 128]
                                       .astype(bf16)),
            "wB": np.ascontiguousarray(
                np.concatenate([Wk[:, 64 * h2:64 * h2 + 64],
                                Wq[:, 64 * h2:64 * h2 + 64]], axis=1)
                .astype(bf16)),
            "wC": np.ascontiguousarray(Wq[:, 64 * h0:64 * h0 + 128]
                                       .astype(bf16)),
            "wV": np.ascontiguousarray(Wv[:, 64 * h0:64 * h0 + HPG * D]
                                       .astype(bf16)),
            "wp01": np.ascontiguousarray(
                W_proj[64 * h0:64 * h0 + 128, :].astype(bf16)),
            "wp2": np.ascontiguousarray(
                W_proj[64 * h2:64 * h2 + 64, :].astype(bf16)),
            "tri": tri,
            "idn": idn,
        }
        if np.any(b_qkv):
            m["bqk"] = np.ascontiguousarray(np.stack([
                bk[64 * h0:64 * h0 + 128],
                np.concatenate([bk[64 * h2:64 * h2 + 64],
                                bq[64 * h2:64 * h2 + 64]]),
                bq[64 * h0:64 * h0 + 128],
            ], axis=1))
            m["bV"] = np.ascontiguousarray(
                bv[None, 64 * h0:64 * h0 + HPG * D].astype(bf16))
        in_maps.append(m)
    return in_maps


def kernel(x, W_qkv, b_qkv, W_proj, b_proj):
    import os
    from concourse.bass_utils import run_bass_kernel_spmd

    in_maps = _prep_inputs(x, W_qkv, b_qkv, W_proj, b_proj)
    with_bias = bool(np.any(np.asarray(b_qkv)))
    key = f"nc{with_bias}"
    if key not in _CACHE:
        _CACHE[key] = _build_program(with_bias)
    nc = _CACHE[key]
    res = run_bass_kernel_spmd(nc, in_maps, list(range(NCORES)),
                               trace=os.environ.get("KTRACE", "") == "1")
    _CACHE["last_result"] = res

    out = np.empty((B, T, C), dtype=np.float32)
    for b in range(B):
        acc = np.zeros((T, C), dtype=np.float32)
        for g in range(4):
            acc += np.asarray(res.results[4 * b + g]["z"],
                              dtype=np.float32)
        out[b] = acc
    bp = np.asarray(b_proj, dtype=np.float32)
    if np.any(bp):
        out += bp[None, None, :]
    return out


# revision 10
# speedup vs baseline: 1.9799x; 1.2054x over previous
"""Causal self-attention (B=2, T=2048, C=768, H=12) on 8 NeuronCores.

Sharding: batch x head-group tensor parallelism (per the hint). Core
d = 4b + g handles batch b and heads [3g, 3g+2]. Each core computes its
3 heads' Q/K/V for the full sequence, the full causal TxT attention for
those heads, and a PARTIAL output projection z_partial = Y_g @ Wproj[g
rows]. The host gather sums the 4 partials per batch (the TP unshard) --
zero device collectives.

On-device pipeline (bf16 operands, fp32 PSUM):
  x^T arrives pre-transposed from host -> QKV^T via W-stationary matmuls
  (K^T/Q^T head-pairs packed 2x64 per 128-partition tile; the odd head's
  K/Q pair is split to base-partition-0 tiles via an SBUF->SBUF DMA).
  S^T[k,q] = (K^T)^T Q^T per 128-k-tile with exact causal q-extent ->
  exp on ACT (scale=1/8, |logits|<=~20 so no max-subtraction) -> upper-
  triangular mask multiply on the diagonal 128-block only -> P.V flipped:
  out O[q,65] per (k-tile, q-tile) with lhsT = P^T block (stationary) and
  rhs = [V|1] -- 65-column outputs make PV ~2x cheaper than the [65,q]
  orientation, and the softmax denominator lands in column 64 so
  normalization is a per-partition broadcast multiply on DVE.

Schedule: V-projection tiles are emitted just-in-time inside head 0's
k-tile loop; Y transposes of heads 0/1 ride inside head 1/2's loops;
the output projection streams per-q-tile behind head 2's per-bank
normalizes. O accumulators pack 7 q-tile windows (72 f32 each) per PSUM
bank: 3 banks for O + 4 for S double-buffering + 1 aux.

PSUM accumulation gotcha: start=True zeroes the full 2KB zero-region
(bank), so only the first window of each bank may issue it.
"""

import numpy as np
import ml_dtypes

B, T, C, H, D = 2, 2048, 768, 12, 64
NCORES = 8
HPG = 3              # heads per group (core)
CT = C // 128        # 6 channel tiles
TT = T // 128        # 16 seq tiles
OSL = 72             # O-window stride (f32) -> 7 windows per 2KB bank
BANKS = ((0, 7), (7, 14), (14, 16))   # O bank -> q-tile ranges

_CACHE = {}


def _build_program(with_bias=True):
    import concourse.bass as bass
    import concourse.bacc as bacc
    import concourse.mybir as mybir
    import concourse.tile as tile

    F32 = mybir.dt.float32
    BF16 = mybir.dt.bfloat16
    AF = mybir.ActivationFunctionType

    nc = bacc.Bacc()
    xTd = nc.declare_dram_parameter("xT", [C, T], BF16, isOutput=False)
    wA = nc.declare_dram_parameter("wA", [C, 128], BF16, isOutput=False)
    wB = nc.declare_dram_parameter("wB", [C, 128], BF16, isOutput=False)
    wC = nc.declare_dram_parameter("wC", [C, 128], BF16, isOutput=False)
    wV = nc.declare_dram_parameter("wV", [C, HPG * D], BF16, isOutput=False)
    wp01 = nc.declare_dram_parameter("wp01", [128, C], BF16, isOutput=False)
    wp2 = nc.declare_dram_parameter("wp2", [64, C], BF16, isOutput=False)
    tri_in = nc.declare_dram_parameter("tri", [128, 128], BF16, isOutput=False)
    idn_in = nc.declare_dram_parameter("idn", [128, 128], BF16, isOutput=False)
    if with_bias:
        bqk = nc.declare_dram_parameter("bqk", [128, 3], F32, isOutput=False)
        bVd = nc.declare_dram_parameter("bV", [1, HPG * D], BF16, isOutput=False)
    z_out = nc.declare_dram_parameter("z", [T, C], BF16, isOutput=True)

    scale = 1.0 / float(np.sqrt(D))

    with tile.TileContext(nc) as tc:
        with tc.tile_pool(name="const", bufs=1) as constp, \
             tc.tile_pool(name="data", bufs=1) as datap, \
             tc.tile_pool(name="pt", bufs=3) as ptp, \
             tc.tile_pool(name="zs", bufs=3) as zsp, \
             tc.tile_pool(name="po", bufs=3, space="PSUM") as pop, \
             tc.tile_pool(name="ps", bufs=2, space="PSUM") as psp, \
             tc.tile_pool(name="aux", bufs=1, space="PSUM") as auxp:

            # ---- constant / weight tiles ---------------------------------
            idn = constp.tile([128, 128], BF16, tag="idn")
            tri = constp.tile([128, 128], BF16, tag="tri")
            wA_s = constp.tile([128, CT, 128], BF16, tag="wA")
            wB_s = constp.tile([128, CT, 128], BF16, tag="wB")
            wC_s = constp.tile([128, CT, 128], BF16, tag="wC")
            wV_s = constp.tile([128, CT, HPG * D], BF16, tag="wV")
            wp01_s = constp.tile([128, C], BF16, tag="wp01")
            wp2_s = constp.tile([64, C], BF16, tag="wp2")
            if with_bias:
                bqk_s = constp.tile([128, 3], F32, tag="bqk")
                bV_s = constp.tile([1, HPG * D], BF16, tag="bV")
                onesB = constp.tile([1, 128], BF16, tag="onesB")
                nc.vector.memset(onesB, 1.0)

            # ---- persistent data tiles -----------------------------------
            xT = [datap.tile([128, T], BF16, tag=f"xT{c}", name=f"xT{c}")
                  for c in range(CT)]
            KT01 = datap.tile([128, T], BF16, tag="KT01", name="KT01")
            QT01 = datap.tile([128, T], BF16, tag="QT01", name="QT01")
            KQ2 = datap.tile([128, T], BF16, tag="KQ2", name="KQ2")
            Q2 = datap.tile([64, T], BF16, tag="Q2", name="Q2")
            V = [datap.tile([128, HPG, D + 1], BF16, tag=f"V{t}",
                            name=f"V{t}") for t in range(TT)]
            Ybf01 = datap.tile([128, TT, 128], BF16, tag="Ybf01", name="Ybf01")
            Ybf2 = datap.tile([128, TT, D], BF16, tag="Ybf2", name="Ybf2")
            YT01 = datap.tile([128, T], BF16, tag="YT01", name="YT01")
            YT2 = datap.tile([64, T], BF16, tag="YT2", name="YT2")
            rec = datap.tile([128, TT, HPG], F32, tag="rec", name="rec")

            # ---- DMAs (ordered by criticality) ---------------------------
            for (w_s, w_d, wid) in ((wC_s, wC, 128), (wA_s, wA, 128)):
                ap = w_d[:, :]
                nc.sync.dma_start(
                    out=w_s,
                    in_=bass.AP(tensor=ap.tensor, offset=ap.offset,
                                ap=[[wid, 128], [128 * wid, CT], [1, wid]]))
            # x^T in quarters so the first QKV chunks start early
            for qq in range(4):
                for c in range(CT):
                    nc.sync.dma_start(
                        out=xT[c][:, 512 * qq:512 * (qq + 1)],
                        in_=xTd[128 * c:128 * (c + 1),
                                512 * qq:512 * (qq + 1)])
                if qq == 0:
                    nc.sync.dma_start(out=tri, in_=tri_in[:, :])
            for (w_s, w_d, wid) in ((wB_s, wB, 128), (wV_s, wV, HPG * D)):
                ap = w_d[:, :]
                nc.gpsimd.dma_start(
                    out=w_s,
                    in_=bass.AP(tensor=ap.tensor, offset=ap.offset,
                                ap=[[wid, 128], [128 * wid, CT], [1, wid]]))
            nc.gpsimd.dma_start(out=idn, in_=idn_in[:, :])
            nc.gpsimd.dma_start(out=wp01_s, in_=wp01[:, :])
            nc.gpsimd.dma_start(out=wp2_s, in_=wp2[:, :])
            if with_bias:
                nc.gpsimd.dma_start(out=bqk_s, in_=bqk[:, :])
                nc.gpsimd.dma_start(out=bV_s, in_=bVd[:, :])

            # ---- QKV^T projection helpers --------------------------------
            def proj_chunk(dst, w_s, bcol, n):
                """dst[:, 512n:512n+512] = chunk of (x @ W_block)^T."""
                acc = psp.tile([128, 512], F32, tag="S", name="acc")
                for c in range(CT):
                    nc.tensor.matmul(
                        out=acc, lhsT=w_s[:, c, :],
                        rhs=xT[c][:, 512 * n:512 * (n + 1)],
                        start=(c == 0), stop=(c == CT - 1))
                if with_bias:
                    nc.vector.tensor_scalar_add(
                        dst[:, 512 * n:512 * (n + 1)], in0=acc,
                        scalar1=bqk_s[:, bcol:bcol + 1])
                else:
                    nc.vector.tensor_copy(
                        out=dst[:, 512 * n:512 * (n + 1)], in_=acc)

            def v_proj(t):
                nc.vector.memset(V[t][:, :, D:D + 1], 1.0)
                acc = auxp.tile([128, 512], F32, tag="aux", name="vacc")
                for c in range(CT):
                    nc.tensor.matmul(
                        out=acc[:, 0:HPG * D],
                        lhsT=xT[c][:, 128 * t:128 * (t + 1)],
                        rhs=wV_s[:, c, :],
                        start=(c == 0), stop=(with_bias is False and
                                              c == CT - 1))
                if with_bias:
                    nc.tensor.matmul(
                        out=acc[:, 0:HPG * D], lhsT=onesB, rhs=bV_s,
                        start=False, stop=True)
                nc.vector.tensor_copy(out=V[t][:, :, 0:D],
                                      in_=acc[:, 0:HPG * D])

            def kq2_chunk(n):
                acc = auxp.tile([128, 512], F32, tag="aux", name="kacc")
                for c in range(CT):
                    nc.tensor.matmul(
                        out=acc, lhsT=wB_s[:, c, :],
                        rhs=xT[c][:, 512 * n:512 * (n + 1)],
                        start=(c == 0), stop=(c == CT - 1))
                if with_bias:
                    nc.vector.tensor_scalar_add(
                        KQ2[:, 512 * n:512 * (n + 1)], in0=acc,
                        scalar1=bqk_s[:, 1:2])
                else:
                    nc.vector.tensor_copy(
                        out=KQ2[:, 512 * n:512 * (n + 1)], in_=acc)

            def transpose_ytile(h, t):
                tp = auxp.tile([128, 128], BF16, tag="aux", name="tp")
                if h == 0:
                    nc.tensor.transpose(out=tp, in_=Ybf01[:, t, :],
                                        identity=idn)
                    nc.vector.tensor_copy(
                        out=YT01[:, 128 * t:128 * (t + 1)], in_=tp)
                else:
                    nc.tensor.transpose(out=tp[0:64, :], in_=Ybf2[:, t, :],
                                        identity=idn)
                    nc.vector.tensor_copy(
                        out=YT2[0:64, 128 * t:128 * (t + 1)],
                        in_=tp[0:64, :])

            def out_proj(t):
                zt = zsp.tile([128, C], BF16, tag="zt", name="zt")
                for (off, w) in ((0, 512), (512, 256)):
                    acc = pop.tile([128, 512], F32, tag="O", name="zacc")
                    nc.tensor.matmul(
                        out=acc[:, 0:w], lhsT=YT01[:, 128 * t:128 * (t + 1)],
                        rhs=wp01_s[:, off:off + w],
                        start=True, stop=False, skip_group_check=True)
                    nc.tensor.matmul(
                        out=acc[:, 0:w], lhsT=YT2[0:64, 128 * t:128 * (t + 1)],
                        rhs=wp2_s[:, off:off + w],
                        start=False, stop=True, skip_group_check=True)
                    if t % 2 == 0:
                        nc.vector.tensor_copy(out=zt[:, off:off + w],
                                              in_=acc[:, 0:w])
                    else:
                        nc.scalar.copy(out=zt[:, off:off + w],
                                       in_=acc[:, 0:w])
                nc.sync.dma_start(
                    out=z_out[128 * t:128 * (t + 1), :], in_=zt)

            # ---- attention -----------------------------------------------
            def normalize_bank(h, obank, b):
                t0, t1 = BANKS[b]
                nsl = t1 - t0
                ob = obank[b]
                rsl = rec[:, t0:t1, h:h + 1]
                nc.vector.reciprocal(out=rsl, in_=ob[:, 0:nsl, D:D + 1])
                rb = bass.AP(tensor=rsl.tensor, offset=rsl.offset,
                             ap=[rsl.ap[0], [HPG, nsl], [0, D]])
                if h < 2:
                    ydst = Ybf01[:, t0:t1, 64 * h:64 * h + D]
                else:
                    ydst = Ybf2[:, t0:t1, 0:D]
                nc.vector.tensor_mul(ydst, ob[:, 0:nsl, 0:D], rb)

            def attention(h, extras):
                KTt, kpo = ((KT01, 0), (KT01, 64), (KQ2, 0))[h]
                QTt, qpo = ((QT01, 0), (QT01, 64), (Q2, 0))[h]
                obank = [None] * 3
                for s in range(TT):
                    for fn in extras.get(s, []):
                        fn()
                    E = T - 128 * s
                    if s == 0:
                        for b in range(3):
                            obank[b] = pop.tile([128, 7, OSL], F32, tag="O",
                                                name=f"O{b}")
                    pt = ptp.tile([128, T], BF16, tag="pt", name="pt")
                    for w0 in range(0, E, 1024):
                        w = min(1024, E - w0)
                        sps = psp.tile([128, 1024], F32, tag="S", name="sps")
                        for half in range(0, w, 512):
                            hw = min(512, w - half)
                            nc.tensor.matmul(
                                out=sps[:, half:half + hw],
                                lhsT=KTt[kpo:kpo + 64, 128 * s:128 * (s + 1)],
                                rhs=QTt[qpo:qpo + 64,
                                        128 * s + w0 + half:
                                        128 * s + w0 + half + hw],
                                start=True, stop=True)
                        nc.scalar.activation(out=pt[:, w0:w0 + w],
                                             in_=sps[:, 0:w],
                                             func=AF.Exp, scale=scale)
                        if w0 == 0:
                            nc.vector.tensor_mul(pt[:, 0:128], pt[:, 0:128],
                                                 tri)
                        # PV for q-tiles covered by this chunk. start=True
                        # zeroes the whole 2KB PSUM zero-region (bank): only
                        # the first window of each bank may issue it.
                        t0 = s + w0 // 128
                        t1 = s + (w0 + w) // 128
                        for t in range(t0, t1):
                            ob = obank[t // 7]
                            nc.tensor.matmul(
                                out=ob[:, t % 7, 0:D + 1],
                                lhsT=pt[:, 128 * (t - s):128 * (t - s) + 128],
                                rhs=V[s][:, h, :],
                                start=(s == 0 and t % 7 == 0), stop=(s == t),
                                skip_group_check=True)
                    for b in range(3):
                        if BANKS[b][1] - 1 == s:
                            normalize_bank(h, obank, b)

            # head 0: V-projection just-in-time (V[s] feeds PV_s), KQ2
            # chunks and remaining QT/KT chunks as PE filler.
            proj_chunk(QT01, wC_s, 2, 0)
            proj_chunk(KT01, wA_s, 0, 0)
            proj_chunk(QT01, wC_s, 2, 1)
            proj_chunk(QT01, wC_s, 2, 2)
            proj_chunk(QT01, wC_s, 2, 3)
            ex0 = {s: [lambda t=s: v_proj(t)] for s in range(TT)}
            ex0[1].append(lambda: proj_chunk(KT01, wA_s, 0, 1))
            ex0[2].append(lambda: proj_chunk(KT01, wA_s, 0, 2))
            ex0[3].append(lambda: proj_chunk(KT01, wA_s, 0, 3))
            ex0[4].append(lambda: kq2_chunk(0))
            ex0[6].append(lambda: kq2_chunk(1))
            ex0[8].append(lambda: kq2_chunk(2))
            ex0[10].append(lambda: kq2_chunk(3))
            ex0[12].append(lambda: nc.sync.dma_start(out=Q2[0:64, :],
                                                     in_=KQ2[64:128, :]))
            attention(0, ex0)

            # head 1: transpose Y of heads 0/1 once both normalized.
            ex1 = {7 + i: [lambda t=i: transpose_ytile(0, t)]
                   for i in range(7)}
            attention(1, ex1)
            for t in range(7, TT):
                transpose_ytile(0, t)

            # head 2: stream output projection behind per-bank normalizes.
            ex2 = {}
            for i in range(7):
                ex2[7 + i] = [lambda t=i: transpose_ytile(2, t)]
                if i >= 1:
                    ex2[7 + i].append(lambda t=i - 1: out_proj(t))
            attention(2, ex2)
            for t in range(7, TT):
                transpose_ytile(2, t)
            for t in range(6, TT):
                out_proj(t)

    nc.finalize()
    return nc


def _prep_inputs(x, W_qkv, b_qkv, W_proj, b_proj):
    bf16 = ml_dtypes.bfloat16
    x = np.asarray(x, dtype=np.float32)
    W_qkv = np.asarray(W_qkv, dtype=np.float32)
    b_qkv = np.asarray(b_qkv, dtype=np.float32)
    W_proj = np.asarray(W_proj, dtype=np.float32)
    b_proj = np.asarray(b_proj, dtype=np.float32)

    Wq, Wk, Wv = W_qkv[:, 0:C], W_qkv[:, C:2 * C], W_qkv[:, 2 * C:3 * C]
    bq, bk, bv = b_qkv[0:C], b_qkv[C:2 * C], b_qkv[2 * C:3 * C]

    xTb = [np.ascontiguousarray(x[b].T.astype(bf16)) for b in range(B)]
    tri = np.triu(np.ones((128, 128), dtype=np.float32)).astype(bf16)
    idn = np.eye(128, dtype=np.float32).astype(bf16)

    in_maps = []
    for d in range(NCORES):
        b, g = d // 4, d % 4
        h0, h2 = 3 * g, 3 * g + 2
        m = {
            "xT": xTb[b],
            "wA": np.ascontiguousarray(Wk[:, 64 * h0:64 * h0 + 128]
                                       .astype(bf16)),
            "wB": np.ascontiguousarray(
                np.concatenate([Wk[:, 64 * h2:64 * h2 + 64],
                                Wq[:, 64 * h2:64 * h2 + 64]], axis=1)
                .astype(bf16)),
            "wC": np.ascontiguousarray(Wq[:, 64 * h0:64 * h0 + 128]
                                       .astype(bf16)),
            "wV": np.ascontiguousarray(Wv[:, 64 * h0:64 * h0 + HPG * D]
                                       .astype(bf16)),
            "wp01": np.ascontiguousarray(
                W_proj[64 * h0:64 * h0 + 128, :].astype(bf16)),
            "wp2": np.ascontiguousarray(
                W_proj[64 * h2:64 * h2 + 64, :].astype(bf16)),
            "tri": tri,
            "idn": idn,
        }
        if np.any(b_qkv):
            m["bqk"] = np.ascontiguousarray(np.stack([
                bk[64 * h0:64 * h0 + 128],
                np.concatenate([bk[64 * h2:64 * h2 + 64],
                                bq[64 * h2:64 * h2 + 64]]),
                bq[64 * h0:64 * h0 + 128],
            ], axis=1))
            m["bV"] = np.ascontiguousarray(
                bv[None, 64 * h0:64 * h0 + HPG * D].astype(bf16))
        in_maps.append(m)
    return in_maps


def kernel(x, W_qkv, b_qkv, W_proj, b_proj):
    import os
    from concourse.bass_utils import run_bass_kernel_spmd

    in_maps = _prep_inputs(x, W_qkv, b_qkv, W_proj, b_proj)
    with_bias = bool(np.any(np.asarray(b_qkv)))
    key = f"nc{with_bias}"
    if key not in _CACHE:
        _CACHE[key] = _build_program(with_bias)
    nc = _CACHE[key]
    res = run_bass_kernel_spmd(nc, in_maps, list(range(NCORES)),
                               trace=os.environ.get("KTRACE", "") == "1")
    _CACHE["last_result"] = res

    out = np.empty((B, T, C), dtype=np.float32)
    for b in range(B):
        acc = np.zeros((T, C), dtype=np.float32)
        for g in range(4):
            acc += np.asarray(res.results[4 * b + g]["z"],
                              dtype=np.float32)
        out[b] = acc
    bp = np.asarray(b_proj, dtype=np.float32)
    if np.any(bp):
        out += bp[None, None, :]
    return out
